# revision 33
# baseline (speedup 1.0000x reference)
"""Multi-head attention (B=4, N=2048, DIM=1024, H=16, HD=64) on 8 TRN2 cores.

Sharding: tensor-parallel over heads — 2 heads per core. The reference omits
the output projection, so each core's output is a disjoint 128-column slice of
the final [B, N, 1024]; no collectives are needed.

Per-core device kernel (bf16 compute, fp32 PSUM accumulation):
  - QKV projection from a single pass over x^T: q^T,k^T produced transposed
    [outch, tokens] (weights stationary), v produced natural [tokens, outch]
    (x tiles stationary) with bias folded in as a K=1 accumulation.
  - scores^T = k^T.T @ q^T per (batch, head): K=64 contraction; head A lives
    on partitions 0-63 and head B on 64-127, so the two heads' matmuls
    row-tile the PE array and run concurrently.
  - exp on ScalarE over two-bank [128, 1024] PSUM tiles -> bf16 SBUF.
  - out^T = [1 | v]^T @ expT accumulated over k tiles (software-pipelined one
    wave deep into the next wave's score stream); row 0 is the softmax
    denominator. Normalization: DVE fast reciprocal of row 0, GpSimd
    partition-broadcast, DVE multiply, DMA out.
"""

import numpy as np
import ml_dtypes

import concourse.bacc as bacc
import concourse.mybir as mybir
from concourse.bass_utils import run_bass_kernel_spmd
from concourse.tile import TileContext

B, N, DIM, H = 4, 2048, 1024, 16
HD = DIM // H
SCALE = 1.0 / np.sqrt(HD)
TOK = B * N               # 8192 tokens
NCORES = 8
HPC = H // NCORES         # heads per core = 2

BF16 = mybir.dt.bfloat16
F32 = mybir.dt.float32
AF = mybir.ActivationFunctionType


NT = TOK // 512           # 16 token tiles of 512 for the projection
KT = 8                    # 1024 / 128 contraction tiles
QT = N // 512             # 4 q tiles per (b, h)
KTOK = N // 128           # 16 k-token tiles per (b, h)
VROW = 2 * (HD + 1)       # 130: [vA | 1 | vB | 1] per token tile


def build_graph():
    nc = bacc.Bacc("TRN2", target_bir_lowering=False, debug=False)
    xt = nc.declare_dram_parameter("xt", [DIM, TOK], BF16, isOutput=False)
    wqk = nc.declare_dram_parameter("wqk", [DIM, 2 * HPC * HD], BF16, isOutput=False)
    wv = nc.declare_dram_parameter("wv", [DIM, HPC * HD], BF16, isOutput=False)
    bqk = nc.declare_dram_parameter("bqk", [2 * HPC * HD, 1], F32, isOutput=False)
    bv = nc.declare_dram_parameter("bv", [1, HPC * HD], BF16, isOutput=False)
    out = nc.declare_dram_parameter("out", [HPC, B, HD, N], F32, isOutput=True)
    NTB = N // 512            # 4 proj token-tiles per batch
    KTOK_B = N // 128         # 16 k-token tiles per batch

    with TileContext(nc) as tc:
        with (
            tc.tile_pool(name="const", bufs=1) as constp,
            tc.tile_pool(name="qk", bufs=1) as qkp,
            tc.tile_pool(name="xin", bufs=3) as xinp,
            tc.tile_pool(name="exps", bufs=36) as expp,
            tc.tile_pool(name="outs", bufs=4) as outp,
            tc.tile_pool(name="rcs", bufs=4) as rcp,
        ):
            # ---- constants ----
            wqk_s = constp.tile([128, KT * 256], BF16)
            for half in range(2):
                nc.sync.dma_start(
                    out=wqk_s[:, half * 1024:(half + 1) * 1024].rearrange(
                        "p (kt j) -> p kt j", kt=KT // 2),
                    in_=wqk.rearrange("(kt p) j -> p kt j", p=128)[
                        :, half * 4:(half + 1) * 4, :])
            wv_s = constp.tile([128, KT * 128], BF16)
            nc.sync.dma_start(
                out=wv_s.rearrange("p (kt j) -> p kt j", kt=KT),
                in_=wv.rearrange("(kt p) j -> p kt j", p=128))
            bqk_s = constp.tile([128, 2], F32)
            for mt in range(2):
                nc.sync.dma_start(out=bqk_s[:, mt:mt + 1],
                                  in_=bqk[mt * 128:(mt + 1) * 128, :])
            bv_s = constp.tile([1, 128], BF16)
            nc.sync.dma_start(out=bv_s[:, :], in_=bv[:, :])
            ones_s = constp.tile([1, 128], BF16)
            nc.vector.memset(ones_s[:, :], 1.0)

            # per-batch activation tensors (lets attention on batch b start
            # as soon as batch b's projection tiles land)
            q_sb = [qkp.tile([128, N], BF16, name=f"q_sb{_b}") for _b in range(B)]
            k_sb = [qkp.tile([128, N], BF16, name=f"k_sb{_b}") for _b in range(B)]
            v_sb = [qkp.tile([128, KTOK_B * VROW], BF16, name=f"v_sb{_b}") for _b in range(B)]
            for _b in range(B):
                nc.vector.memset(v_sb[_b][:, :], 1.0)

            # ---- projection emitted as groups; batches 1-3 interleave into
            # the attention waves' PE slack (scores->av boundary) ----
            with (
                tc.tile_pool(name="pjps", bufs=2, space="PSUM") as pjps,
                tc.tile_pool(name="sps", bufs=2, space="PSUM") as sps,
                tc.tile_pool(name="avps", bufs=1, space="PSUM") as avps,
            ):
                xnt_tiles = {}

                def emit_group(seg):
                    kind = seg[0]
                    if kind == "load":
                        nt = seg[1]
                        xnt = xinp.tile([128, KT * 512], BF16, name="xnt")
                        nc.sync.dma_start(
                            out=xnt.rearrange("p (kt j) -> p kt j", kt=KT),
                            in_=xt.rearrange("(kt p) tok -> p kt tok", p=128)[
                                :, :, nt * 512:(nt + 1) * 512])
                        xnt_tiles[nt] = xnt
                    elif kind == "qk":
                        _, nt, mt = seg
                        bb, ntb = nt // NTB, nt % NTB
                        xnt = xnt_tiles[nt]
                        ps = pjps.tile([128, 512], F32, name="ps", tag="pj")
                        for kt in range(KT):
                            nc.tensor.matmul(
                                ps[:, :],
                                lhsT=wqk_s[:, kt * 256 + mt * 128: kt * 256 + (mt + 1) * 128],
                                rhs=xnt[:, kt * 512:(kt + 1) * 512],
                                start=(kt == 0), stop=(kt == KT - 1))
                        dst = q_sb[bb] if mt == 0 else k_sb[bb]
                        nc.vector.tensor_scalar_add(
                            dst[:, ntb * 512:(ntb + 1) * 512], ps[:, :],
                            bqk_s[:, mt:mt + 1])
                    elif kind == "v":
                        _, nt, sub = seg
                        bb, ntb = nt // NTB, nt % NTB
                        xnt = xnt_tiles[nt]
                        ttb = ntb * 4 + sub
                        vp = pjps.tile([128, 128], F32, name="vp", tag="pj")
                        for kt in range(KT):
                            nc.tensor.matmul(
                                vp[:, :],
                                lhsT=xnt[:, kt * 512 + sub * 128: kt * 512 + (sub + 1) * 128],
                                rhs=wv_s[:, kt * 128:(kt + 1) * 128],
                                start=(kt == 0), stop=False)
                        nc.tensor.matmul(vp[:, :], lhsT=ones_s[:, :], rhs=bv_s[:, :],
                                         start=False, stop=True)
                        nc.vector.tensor_copy(
                            v_sb[bb][:, ttb * VROW + 1: ttb * VROW + 1 + HD],
                            vp[:, 0:HD])
                        nc.vector.tensor_copy(
                            v_sb[bb][:, ttb * VROW + HD + 2: ttb * VROW + 2 * HD + 2],
                            vp[:, HD:2 * HD])

                GCOST = {"load": 0.1, "qk": 2.2, "v": 0.75}

                def proj_groups(bb):
                    segs = []
                    for ntb in range(NTB):
                        nt = bb * NTB + ntb
                        segs.append(("load", nt))
                        for mt in range(2):
                            segs.append(("qk", nt, mt))
                        for sub in range(4):
                            segs.append(("v", nt, sub))
                    return segs

                def emit_tail(pb, pqt, av):
                    for h in range(2):
                        dn = rcp.tile([1, 512], F32, name="dn", tag="dn")
                        nc.vector.tensor_copy(dn[0:1, :], av[h][0:1, :])
                        rc = rcp.tile([1, 512], F32, name="rc", tag="rc")
                        nc.vector.reciprocal_approx_fast(rc[0:1, :], dn[0:1, :])
                        bcs = rcp.tile([65, 512], F32, name="bcs", tag="bcs")
                        nc.gpsimd.partition_broadcast(bcs[:, :], rc[0:1, :])
                        ot = outp.tile([65, 512], F32)
                        nc.vector.tensor_mul(ot[0:65, :], av[h][0:65, :],
                                             bcs[0:65, :])
                        nc.sync.dma_start(
                            out=out[h, pb, :, pqt * 512:(pqt + 1) * 512],
                            in_=ot[1:65, :])

                from collections import deque
                filler = deque()
                for seg in proj_groups(0):
                    emit_group(seg)

                for b in range(B):
                    for qt in range(QT):
                        if qt == 0 and b + 1 < B:
                            filler.extend(proj_groups(b + 1))
                        qcol = qt * 512
                        pav = [avps.tile([65, 512], F32, name=f"av{_h}",
                                         tag=f"av{_h}", bufs=1)
                               for _h in range(2)]
                        echunks = []
                        for kt in range(KTOK_B):
                            kcol = kt * 128
                            s2 = sps.tile([128, 1024], F32, name="s2", tag="s2")
                            for h in range(2):
                                nc.tensor.matmul(
                                    s2[:, h * 512:(h + 1) * 512],
                                    lhsT=k_sb[b][h * 64:(h + 1) * 64, kcol:kcol + 128],
                                    rhs=q_sb[b][h * 64:(h + 1) * 64, qcol:qcol + 512],
                                    start=True, stop=True,
                                    tile_position=(h * 64, 0))
                            e2 = expp.tile([128, 1024], BF16, name="e2", tag="e2")
                            nc.scalar.activation(e2[:, :], s2[:, :], AF.Exp)
                            echunks.append(e2)
                        # wave-boundary projection filler (PE slack while ACT exps)
                        budget = 6.5 if qt < QT - 1 else 1e9
                        while filler and budget > 0:
                            seg = filler.popleft()
                            budget -= GCOST[seg[0]]
                            emit_group(seg)
                        for h in range(2):
                            for kt in range(KTOK_B):
                                nc.tensor.matmul(
                                    pav[h][:, :],
                                    lhsT=v_sb[b][:, kt * VROW + h * (HD + 1): kt * VROW + (h + 1) * (HD + 1)],
                                    rhs=echunks[kt][:, h * 512:(h + 1) * 512],
                                    start=(kt == 0), stop=(kt == KTOK_B - 1),
                                    skip_group_check=True)
                        emit_tail(b, qt, pav)
    nc.compile()
    return nc


_GRAPH = None


def _get_graph():
    global _GRAPH
    if _GRAPH is None:
        _GRAPH = build_graph()
    return _GRAPH


def _make_in_maps(x, w_qkv, b_qkv):
    bf = ml_dtypes.bfloat16
    xt = np.ascontiguousarray(x.reshape(TOK, DIM).T).astype(bf)
    in_maps = []
    for c in range(NCORES):
        hA, hB = HPC * c, HPC * c + 1
        rq = [w_qkv[h * HD:(h + 1) * HD] * SCALE for h in (hA, hB)]
        rk = [w_qkv[DIM + h * HD: DIM + (h + 1) * HD] for h in (hA, hB)]
        rv = [w_qkv[2 * DIM + h * HD: 2 * DIM + (h + 1) * HD] for h in (hA, hB)]
        wqk_c = np.ascontiguousarray(np.concatenate(rq + rk, axis=0).T).astype(bf)
        wv_c = np.ascontiguousarray(np.concatenate(rv, axis=0).T).astype(bf)
        bq = [b_qkv[h * HD:(h + 1) * HD] * SCALE for h in (hA, hB)]
        bk = [b_qkv[DIM + h * HD: DIM + (h + 1) * HD] for h in (hA, hB)]
        bvc = [b_qkv[2 * DIM + h * HD: 2 * DIM + (h + 1) * HD] for h in (hA, hB)]
        bqk_c = np.concatenate(bq + bk).astype(np.float32).reshape(-1, 1)
        bv_c = np.concatenate(bvc).astype(bf).reshape(1, -1)
        in_maps.append({"xt": xt, "wqk": wqk_c, "wv": wv_c,
                        "bqk": np.ascontiguousarray(bqk_c),
                        "bv": np.ascontiguousarray(bv_c)})
    return in_maps


def _run(x, w_qkv, b_qkv, trace=False, tmpdir=None):
    nc = _get_graph()
    in_maps = _make_in_maps(np.asarray(x, dtype=np.float32),
                            np.asarray(w_qkv, dtype=np.float32),
                            np.asarray(b_qkv, dtype=np.float32))
    res = run_bass_kernel_spmd(nc, in_maps, core_ids=list(range(NCORES)),
                               trace=trace, tmpdir=tmpdir)
    full = np.empty((B, N, DIM), dtype=np.float32)
    for c in range(NCORES):
        oc = res.results[c]["out"]          # [HPC, B, HD, N]
        # out[b, q, (HPC*c+hh)*HD + d] = oc[hh, b, d, q]
        full[:, :, c * HPC * HD:(c + 1) * HPC * HD] = \
            oc.transpose(1, 3, 0, 2).reshape(B, N, HPC * HD)
    return full, res


def kernel(x, w_qkv, b_qkv):
    full, _ = _run(x, w_qkv, b_qkv, trace=False)
    return full


# revision 35
# speedup vs baseline: 1.0390x; 1.0390x over previous
"""Multi-head attention (B=4, N=2048, DIM=1024, H=16, HD=64) on 8 TRN2 cores.

Sharding: tensor-parallel over heads — 2 heads per core. The reference omits
the output projection, so each core's output is a disjoint 128-column slice of
the final [B, N, 1024]; no collectives are needed.

Per-core device kernel (bf16 compute, fp32 PSUM accumulation):
  - QKV projection from a single pass over x^T: q^T,k^T produced transposed
    [outch, tokens] (weights stationary), v produced natural [tokens, outch]
    (x tiles stationary) with bias folded in as a K=1 accumulation.
  - scores^T = k^T.T @ q^T per (batch, head): K=64 contraction; head A lives
    on partitions 0-63 and head B on 64-127, so the two heads' matmuls
    row-tile the PE array and run concurrently.
  - exp on ScalarE over two-bank [128, 1024] PSUM tiles -> bf16 SBUF.
  - out^T = [1 | v]^T @ expT accumulated over k tiles (software-pipelined one
    wave deep into the next wave's score stream); row 0 is the softmax
    denominator. Normalization: DVE fast reciprocal of row 0, GpSimd
    partition-broadcast, DVE multiply, DMA out.
"""

import numpy as np
import ml_dtypes

import concourse.bacc as bacc
import concourse.mybir as mybir
from concourse.bass_utils import run_bass_kernel_spmd
from concourse.tile import TileContext

B, N, DIM, H = 4, 2048, 1024, 16
HD = DIM // H
SCALE = 1.0 / np.sqrt(HD)
TOK = B * N               # 8192 tokens
NCORES = 8
HPC = H // NCORES         # heads per core = 2

BF16 = mybir.dt.bfloat16
F32 = mybir.dt.float32
AF = mybir.ActivationFunctionType


NT = TOK // 512           # 16 token tiles of 512 for the projection
KT = 8                    # 1024 / 128 contraction tiles
QT = N // 512             # 4 q tiles per (b, h)
KTOK = N // 128           # 16 k-token tiles per (b, h)
VROW = 2 * (HD + 1)       # 130: [vA | 1 | vB | 1] per token tile


def build_graph():
    nc = bacc.Bacc("TRN2", target_bir_lowering=False, debug=False)
    xt = nc.declare_dram_parameter("xt", [DIM, TOK], BF16, isOutput=False)
    wqk = nc.declare_dram_parameter("wqk", [DIM, 2 * HPC * HD], BF16, isOutput=False)
    wv = nc.declare_dram_parameter("wv", [DIM, HPC * HD], BF16, isOutput=False)
    bqk = nc.declare_dram_parameter("bqk", [2 * HPC * HD, 1], F32, isOutput=False)
    bvq = nc.declare_dram_parameter("bvq", [HD + 1, HPC], F32, isOutput=False)
    out = nc.declare_dram_parameter("out", [HPC, B, HD, N], F32, isOutput=True)
    NTB = N // 512            # 4 proj token-tiles per batch
    KTOK_B = N // 128         # 16 k-token tiles per batch

    with TileContext(nc) as tc:
        with (
            tc.tile_pool(name="const", bufs=1) as constp,
            tc.tile_pool(name="qk", bufs=1) as qkp,
            tc.tile_pool(name="xin", bufs=3) as xinp,
            tc.tile_pool(name="exps", bufs=36) as expp,
            tc.tile_pool(name="outs", bufs=4) as outp,
            tc.tile_pool(name="rcs", bufs=4) as rcp,
        ):
            # ---- constants ----
            wqk_s = constp.tile([128, KT * 256], BF16)
            nc.sync.dma_start(
                out=wqk_s.rearrange("p (kt j) -> p kt j", kt=KT),
                in_=wqk.rearrange("(kt p) j -> p kt j", p=128))
            wv_s = constp.tile([128, KT * 128], BF16)
            nc.sync.dma_start(
                out=wv_s.rearrange("p (kt j) -> p kt j", kt=KT),
                in_=wv.rearrange("(kt p) j -> p kt j", p=128))
            bqk_s = constp.tile([128, 2], F32)
            for mt in range(2):
                nc.sync.dma_start(out=bqk_s[:, mt:mt + 1],
                                  in_=bqk[mt * 128:(mt + 1) * 128, :])
            bvq_s = constp.tile([HD + 1, HPC], F32)
            nc.sync.dma_start(out=bvq_s[:, :], in_=bvq[:, :])

            # per-batch activation tensors (lets attention on batch b start
            # as soon as batch b's projection tiles land)
            q_sb = [qkp.tile([128, N], BF16, name=f"q_sb{_b}") for _b in range(B)]
            k_sb = [qkp.tile([128, N], BF16, name=f"k_sb{_b}") for _b in range(B)]
            v_sb = [qkp.tile([128, KTOK_B * VROW], BF16, name=f"v_sb{_b}") for _b in range(B)]
            for _b in range(B):
                nc.vector.memset(v_sb[_b][:, :], 1.0)

            # ---- projection emitted as groups; batches 1-3 interleave into
            # the attention waves' PE slack (scores->av boundary) ----
            with (
                tc.tile_pool(name="qkps", bufs=1, space="PSUM") as qkps,
                tc.tile_pool(name="vps", bufs=1, space="PSUM") as vps,
                tc.tile_pool(name="sps", bufs=2, space="PSUM") as sps,
                tc.tile_pool(name="avps", bufs=1, space="PSUM") as avps,
            ):
                xnt_tiles = {}

                def emit_group(seg):
                    kind = seg[0]
                    if kind == "load":
                        nt = seg[1]
                        xnt = xinp.tile([128, KT * 512], BF16, name="xnt")
                        nc.sync.dma_start(
                            out=xnt.rearrange("p (kt j) -> p kt j", kt=KT),
                            in_=xt.rearrange("(kt p) tok -> p kt tok", p=128)[
                                :, :, nt * 512:(nt + 1) * 512])
                        xnt_tiles[nt] = xnt
                    elif kind == "qk":
                        _, nt, mt = seg
                        bb, ntb = nt // NTB, nt % NTB
                        xnt = xnt_tiles[nt]
                        ps = qkps.tile([128, 512], F32, name="ps", tag="ps")
                        for kt in range(KT):
                            nc.tensor.matmul(
                                ps[:, :],
                                lhsT=wqk_s[:, kt * 256 + mt * 128: kt * 256 + (mt + 1) * 128],
                                rhs=xnt[:, kt * 512:(kt + 1) * 512],
                                start=(kt == 0), stop=(kt == KT - 1))
                        dst = q_sb[bb] if mt == 0 else k_sb[bb]
                        nc.vector.tensor_scalar_add(
                            dst[:, ntb * 512:(ntb + 1) * 512], ps[:, :],
                            bqk_s[:, mt:mt + 1])
                    elif kind == "v":
                        _, nt, sub = seg
                        bb, ntb = nt // NTB, nt % NTB
                        xnt = xnt_tiles[nt]
                        ttb = ntb * 4 + sub
                        vp = vps.tile([128, 128], F32, name="vp", tag="vp")
                        for kt in range(KT):
                            nc.tensor.matmul(
                                vp[:, :],
                                lhsT=xnt[:, kt * 512 + sub * 128: kt * 512 + (sub + 1) * 128],
                                rhs=wv_s[:, kt * 128:(kt + 1) * 128],
                                start=(kt == 0), stop=(kt == KT - 1))
                        nc.vector.tensor_copy(
                            v_sb[bb][:, ttb * VROW + 1: ttb * VROW + 1 + HD],
                            vp[:, 0:HD])
                        nc.vector.tensor_copy(
                            v_sb[bb][:, ttb * VROW + HD + 2: ttb * VROW + 2 * HD + 2],
                            vp[:, HD:2 * HD])

                GCOST = {"load": 0.1, "qk": 2.2, "v": 0.75}

                def proj_groups(bb):
                    segs = []
                    for ntb in range(NTB):
                        nt = bb * NTB + ntb
                        segs.append(("load", nt))
                        for mt in range(2):
                            segs.append(("qk", nt, mt))
                        for sub in range(4):
                            segs.append(("v", nt, sub))
                    return segs

                def emit_tail(pb, pqt, av):
                    for h in range(2):
                        dn = rcp.tile([1, 512], F32, name="dn", tag="dn")
                        nc.vector.tensor_copy(dn[0:1, :], av[h][0:1, :])
                        rc = rcp.tile([1, 512], F32, name="rc", tag="rc")
                        nc.vector.reciprocal_approx_fast(rc[0:1, :], dn[0:1, :])
                        bcs = rcp.tile([65, 512], F32, name="bcs", tag="bcs")
                        nc.gpsimd.partition_broadcast(bcs[:, :], rc[0:1, :])
                        ot = outp.tile([65, 512], F32)
                        nc.vector.tensor_mul(ot[0:65, :], av[h][0:65, :],
                                             bcs[0:65, :])
                        ot2 = outp.tile([65, 512], F32, name="ot2", tag="ot2")
                        nc.vector.tensor_scalar_add(ot2[0:65, :], ot[0:65, :],
                                                    bvq_s[:, h:h + 1])
                        nc.sync.dma_start(
                            out=out[h, pb, :, pqt * 512:(pqt + 1) * 512],
                            in_=ot2[1:65, :])

                from collections import deque
                filler = deque()
                for seg in proj_groups(0):
                    emit_group(seg)

                for b in range(B):
                    for qt in range(QT):
                        if qt == 0 and b + 1 < B:
                            filler.extend(proj_groups(b + 1))
                        qcol = qt * 512
                        pav = [avps.tile([65, 512], F32, name=f"av{_h}",
                                         tag=f"av{_h}", bufs=1)
                               for _h in range(2)]
                        echunks = []
                        for kt in range(KTOK_B):
                            kcol = kt * 128
                            s2 = sps.tile([128, 1024], F32, name="s2", tag="s2")
                            for h in range(2):
                                nc.tensor.matmul(
                                    s2[:, h * 512:(h + 1) * 512],
                                    lhsT=k_sb[b][h * 64:(h + 1) * 64, kcol:kcol + 128],
                                    rhs=q_sb[b][h * 64:(h + 1) * 64, qcol:qcol + 512],
                                    start=True, stop=True,
                                    tile_position=(h * 64, 0))
                            e2 = expp.tile([128, 1024], BF16, name="e2", tag="e2")
                            nc.scalar.activation(e2[:, :], s2[:, :], AF.Exp)
                            echunks.append(e2)
                        # wave-boundary projection filler (PE slack while ACT exps)
                        budget = 6.5 if qt < QT - 1 else 1e9
                        while filler and budget > 0:
                            seg = filler.popleft()
                            budget -= GCOST[seg[0]]
                            emit_group(seg)
                        for h in range(2):
                            for kt in range(KTOK_B):
                                nc.tensor.matmul(
                                    pav[h][:, :],
                                    lhsT=v_sb[b][:, kt * VROW + h * (HD + 1): kt * VROW + (h + 1) * (HD + 1)],
                                    rhs=echunks[kt][:, h * 512:(h + 1) * 512],
                                    start=(kt == 0), stop=(kt == KTOK_B - 1),
                                    skip_group_check=True)
                        emit_tail(b, qt, pav)
    nc.compile()
    return nc


_GRAPH = None


def _get_graph():
    global _GRAPH
    if _GRAPH is None:
        _GRAPH = build_graph()
    return _GRAPH


def _make_in_maps(x, w_qkv, b_qkv):
    bf = ml_dtypes.bfloat16
    xt = np.ascontiguousarray(x.reshape(TOK, DIM).T).astype(bf)
    in_maps = []
    for c in range(NCORES):
        hA, hB = HPC * c, HPC * c + 1
        rq = [w_qkv[h * HD:(h + 1) * HD] * SCALE for h in (hA, hB)]
        rk = [w_qkv[DIM + h * HD: DIM + (h + 1) * HD] for h in (hA, hB)]
        rv = [w_qkv[2 * DIM + h * HD: 2 * DIM + (h + 1) * HD] for h in (hA, hB)]
        wqk_c = np.ascontiguousarray(np.concatenate(rq + rk, axis=0).T).astype(bf)
        wv_c = np.ascontiguousarray(np.concatenate(rv, axis=0).T).astype(bf)
        bq = [b_qkv[h * HD:(h + 1) * HD] * SCALE for h in (hA, hB)]
        bk = [b_qkv[DIM + h * HD: DIM + (h + 1) * HD] for h in (hA, hB)]
        bvc = [b_qkv[2 * DIM + h * HD: 2 * DIM + (h + 1) * HD] for h in (hA, hB)]
        bqk_c = np.concatenate(bq + bk).astype(np.float32).reshape(-1, 1)
        bvq_c = np.zeros((HD + 1, HPC), dtype=np.float32)
        for hh in range(HPC):
            bvq_c[1:HD + 1, hh] = bvc[hh]
        in_maps.append({"xt": xt, "wqk": wqk_c, "wv": wv_c,
                        "bqk": np.ascontiguousarray(bqk_c),
                        "bvq": bvq_c})
    return in_maps


def _run(x, w_qkv, b_qkv, trace=False, tmpdir=None):
    nc = _get_graph()
    in_maps = _make_in_maps(np.asarray(x, dtype=np.float32),
                            np.asarray(w_qkv, dtype=np.float32),
                            np.asarray(b_qkv, dtype=np.float32))
    res = run_bass_kernel_spmd(nc, in_maps, core_ids=list(range(NCORES)),
                               trace=trace, tmpdir=tmpdir)
    full = np.empty((B, N, DIM), dtype=np.float32)
    for c in range(NCORES):
        oc = res.results[c]["out"]          # [HPC, B, HD, N]
        # out[b, q, (HPC*c+hh)*HD + d] = oc[hh, b, d, q]
        full[:, :, c * HPC * HD:(c + 1) * HPC * HD] = \
            oc.transpose(1, 3, 0, 2).reshape(B, N, HPC * HD)
    return full, res


def kernel(x, w_qkv, b_qkv):
    full, _ = _run(x, w_qkv, b_qkv, trace=False)
    return full
